# revision 6
# baseline (speedup 1.0000x reference)
"""Distributed inverse real SHT on 8 Trainium2 NeuronCores (Bass/Tile).

Math (per reference):
    S[c,k,m]  = sum_l x[c,m,l] * pct[m,k,l]          (Legendre synthesis)
    y[c,k,n]  = irfft_{n=1024}(S, norm='forward')
              = sum_m  Sre[c,k,m]*Fc[m,n] + Sim[c,k,m]*Fs[m,n]
    with Fc[m,n] = w_m cos(2*pi*m*n/N), Fs[m,n] = -w_m sin(2*pi*m*n/N),
    w_0 = 1, w_m = 2 otherwise (verified exactly vs np.fft.irfft).
    pct[m,*,l] = 0 for l < m (triangular), and the m=512 row of pct is
    entirely zero (l < 512 always), so the effective mmax is 512.

Sharding: nlat (k) split across the 8 cores -> 64 output latitudes per
core, no inter-core communication.  Each core streams its own packed
(l >= m) fp16 slice of pct and of x, does per-m-pair matmuls into PSUM,
transposes S via the PE, and applies the DFT matrix as a second matmul.
"""

import numpy as np
from contextlib import ExitStack

NLAT, NLON = 512, 1024
LMAX, MMAX = 512, 513
M_E = 512            # effective mmax (m=512 row of pct is identically zero)
B, C = 1, 16
NCORES = 8
KC = NLAT // NCORES  # 64 latitudes per core
PAIRS = M_E // 2     # 256 m-pairs
TILE_W = 192         # 128 pct cols (2m x 64k) + 64 x cols (2m x 2ri x 16c)
NT_SLAB = 40         # stream tiles per DMA slab
NBLK = 4             # 128-m blocks


def _tiles():
    out = []
    for t in range(PAIRS):
        l0 = 2 * t
        L = LMAX - l0
        nch = (L + 127) // 128
        for c in range(nch):
            out.append((t, l0 + 128 * c, min(128, L - 128 * c), c == nch - 1))
    return out


TILES = _tiles()                 # 640 (pair, l0, K, last_chunk)
NTILES = len(TILES)
F_TOT = NTILES * TILE_W
NSLABS = (NTILES + NT_SLAB - 1) // NT_SLAB

# tiles grouped by PSUM bank: bank G holds pairs 8G..8G+7
_BANK_TILES = [[] for _ in range(PAIRS // 8)]
for _i, (_t, _l0, _K, _lc) in enumerate(TILES):
    _BANK_TILES[_t // 8].append((_i, _t, _l0, _K, _lc))


def build_program():
    from concourse import bacc, bass, masks, mybir, tile

    dt = mybir.dt
    nc = bacc.Bacc("TRN2", target_bir_lowering=False, debug=False,
                   num_devices=NCORES)

    stream = nc.dram_tensor("stream", [128, F_TOT], dt.float16,
                            kind="ExternalInput")
    fmat = nc.dram_tensor("fmat", [128, NBLK * 2 * NLON], dt.float16,
                          kind="ExternalInput")
    y = nc.dram_tensor("y", [C * KC, NLON], dt.float32, kind="ExternalOutput")

    with tile.TileContext(nc) as tc, ExitStack() as ctx:
        sp = ctx.enter_context(tc.tile_pool(name="stream", bufs=2))
        cp = ctx.enter_context(tc.tile_pool(name="const", bufs=1))
        ysp = ctx.enter_context(tc.tile_pool(name="ysb", bufs=3))
        ps1 = ctx.enter_context(
            tc.tile_pool(name="ps1", bufs=2, space=bass.MemorySpace.PSUM))
        pst = ctx.enter_context(
            tc.tile_pool(name="pst", bufs=2, space=bass.MemorySpace.PSUM))
        ps2 = ctx.enter_context(
            tc.tile_pool(name="ps2", bufs=2, space=bass.MemorySpace.PSUM))

        ident = cp.tile([128, 128], dt.float16)
        masks.make_identity(nc, ident[:])

        fsb = cp.tile([128, NBLK * 2 * NLON], dt.float16)
        nc.sync.dma_start(out=fsb[:], in_=fmat[:])
        fsb_v = fsb[:].rearrange("p (b r n) -> p b r n", b=NBLK, r=2, n=NLON)

        # S^T staging: partition = ri*64+k, free = m*16 + c   (fp16)
        snat = cp.tile([128, M_E * C], dt.float16)
        snat_g = snat[:].rearrange("p (g s two c) -> p g s two c",
                                   g=PAIRS // 8, s=8, two=2, c=C)
        snat_m = snat[:].rearrange("p (m c) -> p m c", c=C)

        # stage-2 lhsT: partition = m (mod 128), free = blk*2048 + ri*1024 + c*64 + k
        # (so each stage-2 weight slice [c2 x k64] is one contiguous 128-run)
        lhs = cp.tile([128, NBLK * C * 128], dt.float16)
        lhs_v = lhs[:].rearrange("p (b r c k) -> p b r c k",
                                 b=NBLK, r=2, c=C, k=KC)

        slabs = {}

        def get_slab(j):
            if j not in slabs:
                st = sp.tile([128, NT_SLAB * TILE_W], dt.float16, tag="slab")
                nc.sync.dma_start(
                    out=st[:],
                    in_=stream[:, j * NT_SLAB * TILE_W:(j + 1) * NT_SLAB * TILE_W])
                slabs[j] = st
            return slabs[j]

        # ---- stage 1: Legendre matmuls, 8 m-pairs per PSUM bank ----
        for G in range(PAIRS // 8):
            pb = ps1.tile([128, 512], dt.float32, tag="pb")
            pb_v = pb[:].rearrange("p (s mj r c) -> p s mj r c",
                                   s=8, mj=2, r=2, c=C)
            n_mms = len(_BANK_TILES[G])
            for j, (idx, t, l0, K, lastc) in enumerate(_BANK_TILES[G]):
                st = get_slab(idx // NT_SLAB)
                o = (idx % NT_SLAB) * TILE_W
                s = t % 8
                nc.tensor.matmul(
                    pb[:, s * 64:(s + 1) * 64],
                    st[0:K, o:o + 128],          # lhsT (K x [2m x 64k])
                    st[0:K, o + 128:o + 192],    # rhs  (K x [2m x 2ri x 16c])
                    start=(j == 0), stop=(j == n_mms - 1))
            # extract diagonal (mi==mj) blocks -> snat (cast to fp16)
            for mi in range(2):
                for r in range(2):
                    nc.any.tensor_copy(
                        snat_g[r * 64:(r + 1) * 64, G, :, mi, :],
                        pb_v[mi * 64:(mi + 1) * 64, :, mi, r, :])

            # after each 128-m block (= 8 banks x 16 m) completes: transpose
            if (G + 1) % 8 == 0:
                b = G // 8
                for cc in range(C):
                    pt = pst.tile([128, 128], dt.float16, tag="pt")
                    nc.tensor.transpose(
                        pt[:], snat_m[:, b * 128:(b + 1) * 128, cc], ident[:])
                    # pt cols = ri*64+k; scatter to lhs f = b*2048 + ri*1024 + cc*64 + k
                    nc.any.tensor_copy(lhs_v[:, b, :, cc, :], pt[:])

        # ---- stage 2: DFT matmuls ----
        for s8 in range(8):
            for nch in range(2):
                yp = ps2.tile([128, 512], dt.float32, tag="yp")
                k_i = 0
                for b in range(NBLK):
                    for r in range(2):
                        lo = (b * 2 + r) * C * KC + s8 * 128
                        nc.tensor.matmul(
                            yp[:],
                            lhs[:, lo:lo + 128],
                            fsb_v[:, b, r, nch * 512:(nch + 1) * 512],
                            start=(k_i == 0), stop=(k_i == 2 * NBLK - 1))
                        k_i += 1
                ysb = ysp.tile([128, 512], dt.float32, tag="ysb")
                nc.any.tensor_copy(ysb[:], yp[:])
                nc.sync.dma_start(
                    out=y[s8 * 128:(s8 + 1) * 128, nch * 512:(nch + 1) * 512],
                    in_=ysb[:])

    nc.compile()
    return nc


def _build_fmat():
    m = np.arange(M_E)
    n = np.arange(NLON)
    w = np.where(m == 0, 1.0, 2.0)
    ang = 2.0 * np.pi * np.outer(m, n) / NLON
    Fc = (w[:, None] * np.cos(ang)).astype(np.float16)
    Fs = (-w[:, None] * np.sin(ang)).astype(np.float16)
    fmat = np.zeros((128, NBLK * 2 * NLON), np.float16)
    for b in range(NBLK):
        fmat[:, b * 2 * NLON:b * 2 * NLON + NLON] = Fc[b * 128:(b + 1) * 128]
        fmat[:, b * 2 * NLON + NLON:(b + 1) * 2 * NLON] = Fs[b * 128:(b + 1) * 128]
    return fmat


def _pack_streams(x_re, x_im, pct):
    """Per-core packed fp16 stream: sequence of (128 x 192) tiles."""
    x_re = np.asarray(x_re, np.float32)
    x_im = np.asarray(x_im, np.float32)
    pct = np.asarray(pct, np.float32)

    # x part is core-independent: build once
    xpart = np.zeros((128, NTILES * 64), np.float16)
    for idx, (t, l0, K, _) in enumerate(TILES):
        xr = x_re[0, :, l0:l0 + K, 2 * t:2 * t + 2]   # (c, K, 2m)
        xi = x_im[0, :, l0:l0 + K, 2 * t:2 * t + 2]
        xx = np.stack([xr, xi], axis=0)                # (r, c, K, m)
        xpart[0:K, idx * 64:idx * 64 + 64] = \
            xx.transpose(2, 3, 0, 1).reshape(K, 64)

    streams = []
    for core in range(NCORES):
        k0 = core * KC
        sbuf = np.zeros((128, F_TOT), np.float16)
        sv = sbuf.reshape(128, NTILES, TILE_W)
        sv[:, :, 128:] = xpart.reshape(128, NTILES, 64)
        for idx, (t, l0, K, _) in enumerate(TILES):
            blk = pct[2 * t:2 * t + 2, k0:k0 + KC, l0:l0 + K]  # (2m, 64k, K)
            sv[0:K, idx, 0:128] = blk.transpose(2, 0, 1).reshape(K, 128)
        streams.append(sbuf)
    return streams


_NC_CACHE = [None]


def _get_program():
    if _NC_CACHE[0] is None:
        _NC_CACHE[0] = build_program()
    return _NC_CACHE[0]


def run(x_re, x_im, pct, nlon=NLON, trace=False, trace_kwargs=None):
    from concourse.bass_utils import run_bass_kernel_spmd

    assert int(nlon) == NLON
    nc = _get_program()
    fmat = _build_fmat()
    streams = _pack_streams(x_re, x_im, pct)
    in_maps = [{"stream": streams[i], "fmat": fmat} for i in range(NCORES)]
    res = run_bass_kernel_spmd(nc, in_maps, list(range(NCORES)),
                               trace=trace, **(trace_kwargs or {}))
    out = np.empty((B, C, NLAT, NLON), np.float32)
    for core in range(NCORES):
        yc = res.results[core]["y"].reshape(C, KC, NLON)
        out[0, :, core * KC:(core + 1) * KC, :] = yc
    return out, res


def kernel(x_re, x_im, pct, nlon=NLON, **_unused):
    out, _ = run(x_re, x_im, pct, nlon)
    return out


# revision 7
# speedup vs baseline: 1.0378x; 1.0378x over previous
"""Distributed inverse real SHT on 8 Trainium2 NeuronCores (Bass/Tile).

Math (per reference):
    S[c,k,m]  = sum_l x[c,m,l] * pct[m,k,l]          (Legendre synthesis)
    y[c,k,n]  = irfft_{n=1024}(S, norm='forward')
              = sum_m  Sre[c,k,m]*Fc[m,n] + Sim[c,k,m]*Fs[m,n]
    with Fc[m,n] = w_m cos(2*pi*m*n/N), Fs[m,n] = -w_m sin(2*pi*m*n/N),
    w_0 = 1, w_m = 2 otherwise (verified exactly vs np.fft.irfft).
    pct[m,*,l] = 0 for l < m (triangular), and the m=512 row of pct is
    entirely zero (l < 512 always), so the effective mmax is 512.

Sharding: nlat (k) split across the 8 cores -> 64 output latitudes per
core, no inter-core communication.  Each core streams a packed (l >= m)
fp16 slice of pct and x, does per-m-pair matmuls into PSUM, PE-transposes
S per 128-m block, applies the DFT matmul per block, and accumulates the
block contributions in an SBUF fp32 accumulator (so the DFT work overlaps
the DMA-bound Legendre streaming).
"""

import numpy as np
from contextlib import ExitStack

NLAT, NLON = 512, 1024
LMAX, MMAX = 512, 513
M_E = 512            # effective mmax (m=512 row of pct is identically zero)
B, C = 1, 16
NCORES = 8
KC = NLAT // NCORES  # 64 latitudes per core
PAIRS = M_E // 2     # 256 m-pairs
TILE_W = 192         # 128 pct cols (2m x 64k) + 64 x cols (2m x 2ri x 16c)
NBLK = 4             # 128-m blocks
# graded DMA slab sizes (in tiles): small first so the PE starts early
_SLAB_SIZES = [8, 16, 24, 32] + [40] * 14


def _tiles():
    out = []
    for t in range(PAIRS):
        l0 = 2 * t
        L = LMAX - l0
        nch = (L + 127) // 128
        for c in range(nch):
            out.append((t, l0 + 128 * c, min(128, L - 128 * c), c == nch - 1))
    return out


TILES = _tiles()                 # 640 (pair, l0, K, last_chunk)
NTILES = len(TILES)
assert sum(_SLAB_SIZES) == NTILES
F_TOT = NTILES * TILE_W
# tile idx -> (slab idx, f offset within slab)
_TILE_SLAB = []
for _si, _sz in enumerate(_SLAB_SIZES):
    for _j in range(_sz):
        _TILE_SLAB.append((_si, _j * TILE_W))
_SLAB_OFF = np.cumsum([0] + _SLAB_SIZES)  # in tiles

# tiles grouped by PSUM bank: bank G holds pairs 8G..8G+7
_BANK_TILES = [[] for _ in range(PAIRS // 8)]
for _i, (_t, _l0, _K, _lc) in enumerate(TILES):
    _BANK_TILES[_t // 8].append((_i, _t, _l0, _K, _lc))


def build_program():
    from concourse import bacc, bass, masks, mybir, tile

    dt = mybir.dt
    nc = bacc.Bacc("TRN2", target_bir_lowering=False, debug=False,
                   num_devices=NCORES)

    stream = nc.dram_tensor("stream", [128, F_TOT], dt.float16,
                            kind="ExternalInput")
    fmat = nc.dram_tensor("fmat", [128, NBLK * 2 * NLON], dt.float16,
                          kind="ExternalInput")
    y = nc.dram_tensor("y", [C * KC, NLON], dt.float32, kind="ExternalOutput")

    with tile.TileContext(nc) as tc, ExitStack() as ctx:
        sp = ctx.enter_context(tc.tile_pool(name="stream", bufs=3))
        cp = ctx.enter_context(tc.tile_pool(name="const", bufs=1))
        fp = ctx.enter_context(tc.tile_pool(name="fsb", bufs=2))
        snp = ctx.enter_context(tc.tile_pool(name="snat", bufs=2))
        lhp = ctx.enter_context(tc.tile_pool(name="lhs", bufs=2))
        ps1 = ctx.enter_context(
            tc.tile_pool(name="ps1", bufs=2, space=bass.MemorySpace.PSUM))
        pst = ctx.enter_context(
            tc.tile_pool(name="pst", bufs=2, space=bass.MemorySpace.PSUM))
        ps2 = ctx.enter_context(
            tc.tile_pool(name="ps2", bufs=3, space=bass.MemorySpace.PSUM))

        # fp32 output accumulator: partition = (c2,k64) within strip,
        # free = strip*1024 + n
        acc = cp.tile([128, 8 * NLON], dt.float32)

        slabs = {}

        def get_slab(si):
            if si not in slabs:
                st = sp.tile([128, _SLAB_SIZES[si] * TILE_W], dt.float16,
                             tag="slab")
                o0 = int(_SLAB_OFF[si]) * TILE_W
                nc.sync.dma_start(
                    out=st[:],
                    in_=stream[:, o0:o0 + _SLAB_SIZES[si] * TILE_W])
                slabs[si] = st
            return slabs[si]

        # prefetch first slabs before the (gpsimd) identity setup
        get_slab(0)
        get_slab(1)

        ident = cp.tile([128, 128], dt.float16)
        masks.make_identity(nc, ident[:])

        for b in range(NBLK):
            # DFT matrix slice for this block (prefetched during stage 1)
            fsb = fp.tile([128, 2 * NLON], dt.float16, tag="fsb")
            nc.sync.dma_start(
                out=fsb[:], in_=fmat[:, b * 2 * NLON:(b + 1) * 2 * NLON])
            fsb_v = fsb[:].rearrange("p (r n) -> p r n", r=2, n=NLON)

            # S^T staging for this 128-m block:
            #   partition = ri*64+k, free = m_loc*16 + c   (fp16)
            snat = snp.tile([128, 128 * C], dt.float16, tag="snat")
            snat_g = snat[:].rearrange("p (g s two c) -> p g s two c",
                                       g=8, s=8, two=2, c=C)
            snat_m = snat[:].rearrange("p (m c) -> p m c", c=C)

            # ---- stage 1: Legendre matmuls, 8 m-pairs per PSUM bank ----
            for g in range(8):
                G = b * 8 + g
                pb = ps1.tile([128, 512], dt.float32, tag="pb")
                pb_v = pb[:].rearrange("p (s mj r c) -> p s mj r c",
                                       s=8, mj=2, r=2, c=C)
                n_mms = len(_BANK_TILES[G])
                for j, (idx, t, l0, K, lastc) in enumerate(_BANK_TILES[G]):
                    si, o = _TILE_SLAB[idx]
                    st = get_slab(si)
                    if si + 1 < len(_SLAB_SIZES):
                        get_slab(si + 1)  # prefetch
                    s = t % 8
                    nc.tensor.matmul(
                        pb[:, s * 64:(s + 1) * 64],
                        st[0:K, o:o + 128],          # lhsT (K x [2m x 64k])
                        st[0:K, o + 128:o + 192],    # rhs (K x [2m x 2ri x 16c])
                        start=(j == 0), stop=(j == n_mms - 1))
                # extract diagonal (mi==mj) blocks -> snat (cast fp16), DVE
                for mi in range(2):
                    for r in range(2):
                        nc.vector.tensor_copy(
                            snat_g[r * 64:(r + 1) * 64, g, :, mi, :],
                            pb_v[mi * 64:(mi + 1) * 64, :, mi, r, :])

            # ---- transpose S block into stage-2 lhsT layout ----
            # lhs: partition = m_loc, free = ri*1024 + c*64 + k
            lhs = lhp.tile([128, 2 * C * KC], dt.float16, tag="lhs")
            lhs_v = lhs[:].rearrange("p (r c k) -> p r c k", r=2, c=C, k=KC)
            for cc in range(C):
                pt = pst.tile([128, 128], dt.float16, tag="pt")
                nc.tensor.transpose(
                    pt[:], snat_m[:, :, cc], ident[:])
                # pt cols = ri*64+k -> lhs f = ri*1024 + cc*64 + k
                nc.scalar.copy(lhs_v[:, :, cc, :], pt[:])

            # ---- stage 2: DFT matmuls for this block, accumulate in SBUF ----
            for s8 in range(8):
                for nch in range(2):
                    yp = ps2.tile([128, 512], dt.float32, tag="yp")
                    for r in range(2):
                        nc.tensor.matmul(
                            yp[:],
                            lhs[:, r * C * KC + s8 * 128:
                                r * C * KC + s8 * 128 + 128],
                            fsb_v[:, r, nch * 512:(nch + 1) * 512],
                            start=(r == 0), stop=(r == 1))
                    a_sl = acc[:, s8 * NLON + nch * 512:
                               s8 * NLON + nch * 512 + 512]
                    if b == 0:
                        nc.vector.tensor_copy(a_sl, yp[:])
                    else:
                        nc.vector.tensor_add(a_sl, a_sl, yp[:])

        # ---- write out ----
        for s8 in range(8):
            nc.sync.dma_start(
                out=y[s8 * 128:(s8 + 1) * 128, :],
                in_=acc[:, s8 * NLON:(s8 + 1) * NLON])

    nc.compile()
    return nc


def _build_fmat():
    m = np.arange(M_E)
    n = np.arange(NLON)
    w = np.where(m == 0, 1.0, 2.0)
    ang = 2.0 * np.pi * np.outer(m, n) / NLON
    Fc = (w[:, None] * np.cos(ang)).astype(np.float16)
    Fs = (-w[:, None] * np.sin(ang)).astype(np.float16)
    fmat = np.zeros((128, NBLK * 2 * NLON), np.float16)
    for b in range(NBLK):
        fmat[:, b * 2 * NLON:b * 2 * NLON + NLON] = Fc[b * 128:(b + 1) * 128]
        fmat[:, b * 2 * NLON + NLON:(b + 1) * 2 * NLON] = Fs[b * 128:(b + 1) * 128]
    return fmat


def _pack_streams(x_re, x_im, pct):
    """Per-core packed fp16 stream: sequence of (128 x 192) tiles."""
    x_re = np.asarray(x_re, np.float32)
    x_im = np.asarray(x_im, np.float32)
    pct = np.asarray(pct, np.float32)

    # x part is core-independent: build once
    xpart = np.zeros((128, NTILES * 64), np.float16)
    for idx, (t, l0, K, _) in enumerate(TILES):
        xr = x_re[0, :, l0:l0 + K, 2 * t:2 * t + 2]   # (c, K, 2m)
        xi = x_im[0, :, l0:l0 + K, 2 * t:2 * t + 2]
        xx = np.stack([xr, xi], axis=0)                # (r, c, K, m)
        xpart[0:K, idx * 64:idx * 64 + 64] = \
            xx.transpose(2, 3, 0, 1).reshape(K, 64)

    streams = []
    for core in range(NCORES):
        k0 = core * KC
        sbuf = np.zeros((128, F_TOT), np.float16)
        sv = sbuf.reshape(128, NTILES, TILE_W)
        sv[:, :, 128:] = xpart.reshape(128, NTILES, 64)
        for idx, (t, l0, K, _) in enumerate(TILES):
            blk = pct[2 * t:2 * t + 2, k0:k0 + KC, l0:l0 + K]  # (2m, 64k, K)
            sv[0:K, idx, 0:128] = blk.transpose(2, 0, 1).reshape(K, 128)
        streams.append(sbuf)
    return streams


_NC_CACHE = [None]


def _get_program():
    if _NC_CACHE[0] is None:
        _NC_CACHE[0] = build_program()
    return _NC_CACHE[0]


def run(x_re, x_im, pct, nlon=NLON, trace=False, trace_kwargs=None):
    from concourse.bass_utils import run_bass_kernel_spmd

    assert int(nlon) == NLON
    nc = _get_program()
    fmat = _build_fmat()
    streams = _pack_streams(x_re, x_im, pct)
    in_maps = [{"stream": streams[i], "fmat": fmat} for i in range(NCORES)]
    res = run_bass_kernel_spmd(nc, in_maps, list(range(NCORES)),
                               trace=trace, **(trace_kwargs or {}))
    out = np.empty((B, C, NLAT, NLON), np.float32)
    for core in range(NCORES):
        yc = res.results[core]["y"].reshape(C, KC, NLON)
        out[0, :, core * KC:(core + 1) * KC, :] = yc
    return out, res


def kernel(x_re, x_im, pct, nlon=NLON, **_unused):
    out, _ = run(x_re, x_im, pct, nlon)
    return out


# revision 11
# speedup vs baseline: 1.1297x; 1.0885x over previous
"""Distributed inverse real SHT on 8 Trainium2 NeuronCores (Bass/Tile).

Math (per reference):
    S[c,k,m]  = sum_l x[c,m,l] * pct[m,k,l]          (Legendre synthesis)
    y[c,k,n]  = irfft_{n=1024}(S, norm='forward')
              = sum_m  Sre[c,k,m]*Fc[m,n] + Sim[c,k,m]*Fs[m,n]
    with Fc[m,n] = w_m cos(2*pi*m*n/N), Fs[m,n] = -w_m sin(2*pi*m*n/N),
    w_0 = 1, w_m = 2 otherwise (verified exactly vs np.fft.irfft).
    pct[m,*,l] = 0 for l < m (triangular), and the m=512 row of pct is
    entirely zero (l < 512 always), so the effective mmax is 512.

Sharding: nlat (k) split across the 8 cores -> 64 output latitudes per
core, no inter-core communication.  Each core streams a packed (l >= m)
fp16 slice of pct and x, does per-m-pair matmuls into PSUM, PE-transposes
S per 128-m block, applies the DFT matmul per block, and accumulates the
block contributions in an SBUF fp32 accumulator (so the DFT work overlaps
the DMA-bound Legendre streaming).
"""

import numpy as np
from contextlib import ExitStack




NLAT, NLON = 512, 1024
LMAX, MMAX = 512, 513
M_E = 512            # effective mmax (m=512 row of pct is identically zero)
B, C = 1, 16
NCORES = 8
KC = NLAT // NCORES  # 64 latitudes per core
PAIRS = M_E // 2     # 256 m-pairs
TILE_W = 192         # 128 pct cols (2m x 64k) + 64 x cols (2m x 2ri x 16c)
NBLK = 4             # 128-m blocks
# graded DMA slab sizes (in tiles): small first so the PE starts early
_SLAB_SIZES = [8, 8] + [16] * 39


def _tiles():
    out = []
    for t in range(PAIRS):
        l0 = 2 * t
        L = LMAX - l0
        nch = (L + 127) // 128
        for c in range(nch):
            out.append((t, l0 + 128 * c, min(128, L - 128 * c), c == nch - 1))
    return out


TILES = _tiles()                 # 640 (pair, l0, K, last_chunk)
NTILES = len(TILES)
assert sum(_SLAB_SIZES) == NTILES
F_TOT = NTILES * TILE_W
# tile idx -> (slab idx, f offset within slab)
_TILE_SLAB = []
for _si, _sz in enumerate(_SLAB_SIZES):
    for _j in range(_sz):
        _TILE_SLAB.append((_si, _j * TILE_W))
_SLAB_OFF = np.cumsum([0] + _SLAB_SIZES)  # in tiles

# tiles grouped by PSUM bank: bank G holds pairs 8G..8G+7
_BANK_TILES = [[] for _ in range(PAIRS // 8)]
for _i, (_t, _l0, _K, _lc) in enumerate(TILES):
    _BANK_TILES[_t // 8].append((_i, _t, _l0, _K, _lc))


def build_program():
    from concourse import bacc, bass, masks, mybir, tile

    dt = mybir.dt
    nc = bacc.Bacc("TRN2", target_bir_lowering=False, debug=False,
                   num_devices=NCORES)

    stream = nc.dram_tensor("stream", [128, F_TOT], dt.float16,
                            kind="ExternalInput")
    fmat = nc.dram_tensor("fmat", [128, NBLK * 2 * NLON], dt.float16,
                          kind="ExternalInput")
    y = nc.dram_tensor("y", [C * KC, NLON], dt.float32, kind="ExternalOutput")

    with tile.TileContext(nc) as tc, ExitStack() as ctx:
        sp = ctx.enter_context(tc.tile_pool(name="stream", bufs=4))
        cp = ctx.enter_context(tc.tile_pool(name="const", bufs=1))
        fp = ctx.enter_context(tc.tile_pool(name="fsb", bufs=2))
        snp = ctx.enter_context(tc.tile_pool(name="snat", bufs=2))
        lhp = ctx.enter_context(tc.tile_pool(name="lhs", bufs=2))
        ps1 = ctx.enter_context(
            tc.tile_pool(name="ps1", bufs=2, space=bass.MemorySpace.PSUM))
        pst = ctx.enter_context(
            tc.tile_pool(name="pst", bufs=2, space=bass.MemorySpace.PSUM))
        ps2 = ctx.enter_context(
            tc.tile_pool(name="ps2", bufs=2, space=bass.MemorySpace.PSUM))

        # fp32 output accumulator: partition = (c2,k64) within strip,
        # free = strip*1024 + n
        acc = cp.tile([128, 8 * NLON], dt.float32)

        slabs = {}

        def get_slab(si):
            if si not in slabs:
                st = sp.tile([128, _SLAB_SIZES[si] * TILE_W], dt.float16,
                             tag="slab")
                o0 = int(_SLAB_OFF[si]) * TILE_W
                nc.sync.dma_start(
                    out=st[:],
                    in_=stream[:, o0:o0 + _SLAB_SIZES[si] * TILE_W])
                slabs[si] = st
            return slabs[si]

        # prefetch first slabs before the (gpsimd) identity setup
        get_slab(0)
        get_slab(1)

        ident = cp.tile([128, 128], dt.float16)
        masks.make_identity(nc, ident[:])

        deferred = []  # previous block's transpose + DFT work, as thunks

        def make_deferred(b, snat_m, fsb_v):
            """Block b's post-stage-1 work: PE transposes into the stage-2
            lhsT layout, then the DFT matmuls + SBUF accumulation.  Emitted
            interleaved into block b+1's stage-1 stream so the in-order PE
            never stalls on the DVE extract chain."""
            thunks = []
            lhs = lhp.tile([128, 2 * C * KC], dt.float16, tag="lhs")
            lhs_v = lhs[:].rearrange("p (r c k) -> p r c k", r=2, c=C, k=KC)

            def transp(cc):
                pt = pst.tile([128, 128], dt.float16, tag="pt")
                nc.tensor.transpose(pt[:], snat_m[:, :, cc], ident[:])
                # pt cols = ri*64+k -> lhs f = ri*1024 + cc*64 + k
                eng = nc.scalar.copy if cc % 2 else nc.vector.tensor_copy
                eng(lhs_v[:, :, cc, :], pt[:])

            for cc in range(C):
                thunks.append(lambda cc=cc: transp(cc))

            def dft(s8):
                yp = ps2.tile([128, 1024], dt.float32, tag="yp")
                for nch in range(2):
                    for r in range(2):
                        nc.tensor.matmul(
                            yp[:, nch * 512:(nch + 1) * 512],
                            lhs[:, r * C * KC + s8 * 128:
                                r * C * KC + s8 * 128 + 128],
                            fsb_v[:, r, nch * 512:(nch + 1) * 512],
                            start=(r == 0), stop=(r == 1))
                a_sl = acc[:, s8 * NLON:(s8 + 1) * NLON]
                if b == 0:
                    nc.vector.tensor_copy(a_sl, yp[:])
                else:
                    nc.vector.tensor_add(a_sl, a_sl, yp[:])
                if b == NBLK - 1:
                    nc.sync.dma_start(
                        out=y[s8 * 128:(s8 + 1) * 128, :], in_=a_sl)

            for s8 in range(8):
                thunks.append(lambda s8=s8: dft(s8))
            return thunks

        for b in range(NBLK):
            # DFT matrix slice for this block (prefetched during stage 1)
            fsb = fp.tile([128, 2 * NLON], dt.float16, tag="fsb")
            nc.sync.dma_start(
                out=fsb[:], in_=fmat[:, b * 2 * NLON:(b + 1) * 2 * NLON])
            fsb_v = fsb[:].rearrange("p (r n) -> p r n", r=2, n=NLON)

            # S^T staging for this 128-m block:
            #   partition = ri*64+k, free = m_loc*16 + c   (fp16)
            snat = snp.tile([128, 128 * C], dt.float16, tag="snat")
            snat_g = snat[:].rearrange("p (g s two c) -> p g s two c",
                                       g=8, s=8, two=2, c=C)
            snat_m = snat[:].rearrange("p (m c) -> p m c", c=C)

            # ---- stage 1: Legendre matmuls, 8 m-pairs per PSUM bank ----
            for g in range(8):
                G = b * 8 + g
                pb = ps1.tile([128, 512], dt.float32, tag="pb")
                pb_v = pb[:].rearrange("p (s mj r c) -> p s mj r c",
                                       s=8, mj=2, r=2, c=C)
                n_mms = len(_BANK_TILES[G])
                for j, (idx, t, l0, K, lastc) in enumerate(_BANK_TILES[G]):
                    si, o = _TILE_SLAB[idx]
                    st = get_slab(si)
                    if si + 1 < len(_SLAB_SIZES):
                        get_slab(si + 1)  # prefetch
                    s = t % 8
                    nc.tensor.matmul(
                        pb[:, s * 64:(s + 1) * 64],
                        st[0:K, o:o + 128],          # lhsT (K x [2m x 64k])
                        st[0:K, o + 128:o + 192],    # rhs (K x [2m x 2ri x 16c])
                        start=(j == 0), stop=(j == n_mms - 1))
                # extract diagonal (mi==mj) blocks -> snat (cast fp16),
                # split across DVE and ACT
                for mi in range(2):
                    for r in range(2):
                        eng = (nc.vector.tensor_copy if (mi + r) % 2 == 0
                               else nc.scalar.copy)
                        eng(snat_g[r * 64:(r + 1) * 64, g, :, mi, :],
                            pb_v[mi * 64:(mi + 1) * 64, :, mi, r, :])
                # interleave ~3 deferred units from the previous block
                for _ in range(3):
                    if deferred:
                        deferred.pop(0)()

            while deferred:
                deferred.pop(0)()
            deferred = make_deferred(b, snat_m, fsb_v)

        # last block's work has no next block to hide in
        while deferred:
            deferred.pop(0)()

    nc.compile()
    return nc


def _build_fmat():
    m = np.arange(M_E)
    n = np.arange(NLON)
    w = np.where(m == 0, 1.0, 2.0)
    ang = 2.0 * np.pi * np.outer(m, n) / NLON
    Fc = (w[:, None] * np.cos(ang)).astype(np.float16)
    Fs = (-w[:, None] * np.sin(ang)).astype(np.float16)
    fmat = np.zeros((128, NBLK * 2 * NLON), np.float16)
    for b in range(NBLK):
        fmat[:, b * 2 * NLON:b * 2 * NLON + NLON] = Fc[b * 128:(b + 1) * 128]
        fmat[:, b * 2 * NLON + NLON:(b + 1) * 2 * NLON] = Fs[b * 128:(b + 1) * 128]
    return fmat


def _pack_streams(x_re, x_im, pct):
    """Per-core packed fp16 stream: sequence of (128 x 192) tiles."""
    x_re = np.asarray(x_re, np.float32)
    x_im = np.asarray(x_im, np.float32)
    pct = np.asarray(pct, np.float32)

    # x part is core-independent: build once
    xpart = np.zeros((128, NTILES * 64), np.float16)
    for idx, (t, l0, K, _) in enumerate(TILES):
        xr = x_re[0, :, l0:l0 + K, 2 * t:2 * t + 2]   # (c, K, 2m)
        xi = x_im[0, :, l0:l0 + K, 2 * t:2 * t + 2]
        xx = np.stack([xr, xi], axis=0)                # (r, c, K, m)
        xpart[0:K, idx * 64:idx * 64 + 64] = \
            xx.transpose(2, 3, 0, 1).reshape(K, 64)

    streams = []
    for core in range(NCORES):
        k0 = core * KC
        sbuf = np.zeros((128, F_TOT), np.float16)
        sv = sbuf.reshape(128, NTILES, TILE_W)
        sv[:, :, 128:] = xpart.reshape(128, NTILES, 64)
        for idx, (t, l0, K, _) in enumerate(TILES):
            blk = pct[2 * t:2 * t + 2, k0:k0 + KC, l0:l0 + K]  # (2m, 64k, K)
            sv[0:K, idx, 0:128] = blk.transpose(2, 0, 1).reshape(K, 128)
        streams.append(sbuf)
    return streams


_NC_CACHE = [None]


def _get_program():
    if _NC_CACHE[0] is None:
        _NC_CACHE[0] = build_program()
    return _NC_CACHE[0]


def run(x_re, x_im, pct, nlon=NLON, trace=False, trace_kwargs=None):
    from concourse.bass_utils import run_bass_kernel_spmd

    assert int(nlon) == NLON
    nc = _get_program()
    fmat = _build_fmat()
    streams = _pack_streams(x_re, x_im, pct)
    in_maps = [{"stream": streams[i], "fmat": fmat} for i in range(NCORES)]
    res = run_bass_kernel_spmd(nc, in_maps, list(range(NCORES)),
                               trace=trace, **(trace_kwargs or {}))
    out = np.empty((B, C, NLAT, NLON), np.float32)
    for core in range(NCORES):
        yc = res.results[core]["y"].reshape(C, KC, NLON)
        out[0, :, core * KC:(core + 1) * KC, :] = yc
    return out, res


def kernel(x_re, x_im, pct, nlon=NLON, **_unused):
    out, _ = run(x_re, x_im, pct, nlon)
    return out


# revision 12
# speedup vs baseline: 1.1383x; 1.0076x over previous
"""Distributed inverse real SHT on 8 Trainium2 NeuronCores (Bass/Tile).

Math (per reference):
    S[c,k,m]  = sum_l x[c,m,l] * pct[m,k,l]          (Legendre synthesis)
    y[c,k,n]  = irfft_{n=1024}(S, norm='forward')
              = sum_m  Sre[c,k,m]*Fc[m,n] + Sim[c,k,m]*Fs[m,n]
    with Fc[m,n] = w_m cos(2*pi*m*n/N), Fs[m,n] = -w_m sin(2*pi*m*n/N),
    w_0 = 1, w_m = 2 otherwise (verified exactly vs np.fft.irfft).
    pct[m,*,l] = 0 for l < m (triangular), and the m=512 row of pct is
    entirely zero (l < 512 always), so the effective mmax is 512.

Sharding: nlat (k) split across the 8 cores -> 64 output latitudes per
core, no inter-core communication.  Each core streams a packed (l >= m)
fp16 slice of pct and x, does per-m-pair matmuls into PSUM, PE-transposes
S per 128-m block, applies the DFT matmul per block, and accumulates the
block contributions in an SBUF fp32 accumulator (so the DFT work overlaps
the DMA-bound Legendre streaming).
"""

import numpy as np
from contextlib import ExitStack




NLAT, NLON = 512, 1024
LMAX, MMAX = 512, 513
M_E = 512            # effective mmax (m=512 row of pct is identically zero)
B, C = 1, 16
NCORES = 8
KC = NLAT // NCORES  # 64 latitudes per core
PAIRS = M_E // 2     # 256 m-pairs
TILE_W = 192         # 128 pct cols (2m x 64k) + 64 x cols (2m x 2ri x 16c)
NBLK = 4             # 128-m blocks
# graded DMA slab sizes (in tiles): small first so the PE starts early
_SLAB_SIZES = [8, 8] + [16] * 39


def _tiles():
    out = []
    for t in range(PAIRS):
        l0 = 2 * t
        L = LMAX - l0
        nch = (L + 127) // 128
        for c in range(nch):
            out.append((t, l0 + 128 * c, min(128, L - 128 * c), c == nch - 1))
    return out


TILES = _tiles()                 # 640 (pair, l0, K, last_chunk)
NTILES = len(TILES)
assert sum(_SLAB_SIZES) == NTILES
F_TOT = NTILES * TILE_W
# tile idx -> (slab idx, f offset within slab)
_TILE_SLAB = []
for _si, _sz in enumerate(_SLAB_SIZES):
    for _j in range(_sz):
        _TILE_SLAB.append((_si, _j * TILE_W))
_SLAB_OFF = np.cumsum([0] + _SLAB_SIZES)  # in tiles

# tiles grouped by PSUM bank: bank G holds pairs 8G..8G+7
_BANK_TILES = [[] for _ in range(PAIRS // 8)]
for _i, (_t, _l0, _K, _lc) in enumerate(TILES):
    _BANK_TILES[_t // 8].append((_i, _t, _l0, _K, _lc))


def build_program():
    from concourse import bacc, bass, masks, mybir, tile

    dt = mybir.dt
    nc = bacc.Bacc("TRN2", target_bir_lowering=False, debug=False,
                   num_devices=NCORES)

    stream = nc.dram_tensor("stream", [128, F_TOT], dt.float16,
                            kind="ExternalInput")
    fmat = nc.dram_tensor("fmat", [128, NBLK * 2 * NLON], dt.float16,
                          kind="ExternalInput")
    y = nc.dram_tensor("y", [C * KC, NLON], dt.float32, kind="ExternalOutput")

    with tile.TileContext(nc) as tc, ExitStack() as ctx:
        sp = ctx.enter_context(tc.tile_pool(name="stream", bufs=8))
        cp = ctx.enter_context(tc.tile_pool(name="const", bufs=1))
        fp = ctx.enter_context(tc.tile_pool(name="fsb", bufs=2))
        snp = ctx.enter_context(tc.tile_pool(name="snat", bufs=2))
        lhp = ctx.enter_context(tc.tile_pool(name="lhs", bufs=2))
        ps1 = ctx.enter_context(
            tc.tile_pool(name="ps1", bufs=2, space=bass.MemorySpace.PSUM))
        pst = ctx.enter_context(
            tc.tile_pool(name="pst", bufs=2, space=bass.MemorySpace.PSUM))
        ps2 = ctx.enter_context(
            tc.tile_pool(name="ps2", bufs=2, space=bass.MemorySpace.PSUM))

        # fp32 output accumulator: partition = (c2,k64) within strip,
        # free = strip*1024 + n
        acc = cp.tile([128, 8 * NLON], dt.float32)

        slabs = {}

        def get_slab(si):
            if si not in slabs:
                st = sp.tile([128, _SLAB_SIZES[si] * TILE_W], dt.float16,
                             tag="slab")
                o0 = int(_SLAB_OFF[si]) * TILE_W
                nc.sync.dma_start(
                    out=st[:],
                    in_=stream[:, o0:o0 + _SLAB_SIZES[si] * TILE_W])
                slabs[si] = st
            return slabs[si]

        # prefetch first slabs before the (gpsimd) identity setup
        get_slab(0)
        get_slab(1)

        ident = cp.tile([128, 128], dt.float16)
        masks.make_identity(nc, ident[:])

        deferred = []  # previous block's transpose + DFT work, as thunks

        def make_deferred(b, snat_m, fsb_v):
            """Block b's post-stage-1 work: PE transposes into the stage-2
            lhsT layout, then the DFT matmuls + SBUF accumulation.  Emitted
            interleaved into block b+1's stage-1 stream so the in-order PE
            never stalls on the DVE extract chain."""
            thunks = []
            lhs = lhp.tile([128, 2 * C * KC], dt.float16, tag="lhs")
            lhs_v = lhs[:].rearrange("p (r c k) -> p r c k", r=2, c=C, k=KC)

            def transp(cc):
                pt = pst.tile([128, 128], dt.float16, tag="pt")
                nc.tensor.transpose(pt[:], snat_m[:, :, cc], ident[:])
                # pt cols = ri*64+k -> lhs f = ri*1024 + cc*64 + k
                eng = nc.scalar.copy if cc % 2 else nc.vector.tensor_copy
                eng(lhs_v[:, :, cc, :], pt[:])

            for cc in range(C):
                thunks.append(lambda cc=cc: transp(cc))

            def dft(s8):
                yp = ps2.tile([128, 1024], dt.float32, tag="yp")
                for nch in range(2):
                    for r in range(2):
                        nc.tensor.matmul(
                            yp[:, nch * 512:(nch + 1) * 512],
                            lhs[:, r * C * KC + s8 * 128:
                                r * C * KC + s8 * 128 + 128],
                            fsb_v[:, r, nch * 512:(nch + 1) * 512],
                            start=(r == 0), stop=(r == 1))
                a_sl = acc[:, s8 * NLON:(s8 + 1) * NLON]
                if b == 0:
                    nc.vector.tensor_copy(a_sl, yp[:])
                else:
                    nc.vector.tensor_add(a_sl, a_sl, yp[:])
                if b == NBLK - 1:
                    nc.sync.dma_start(
                        out=y[s8 * 128:(s8 + 1) * 128, :], in_=a_sl)

            for s8 in range(8):
                thunks.append(lambda s8=s8: dft(s8))
            return thunks

        for b in range(NBLK):
            # DFT matrix slice for this block (prefetched during stage 1)
            fsb = fp.tile([128, 2 * NLON], dt.float16, tag="fsb")
            nc.sync.dma_start(
                out=fsb[:], in_=fmat[:, b * 2 * NLON:(b + 1) * 2 * NLON])
            fsb_v = fsb[:].rearrange("p (r n) -> p r n", r=2, n=NLON)

            # S^T staging for this 128-m block:
            #   partition = ri*64+k, free = m_loc*16 + c   (fp16)
            snat = snp.tile([128, 128 * C], dt.float16, tag="snat")
            snat_g = snat[:].rearrange("p (g s two c) -> p g s two c",
                                       g=8, s=8, two=2, c=C)
            snat_m = snat[:].rearrange("p (m c) -> p m c", c=C)

            # ---- stage 1: Legendre matmuls, 8 m-pairs per PSUM bank ----
            for g in range(8):
                G = b * 8 + g
                pb = ps1.tile([128, 512], dt.float32, tag="pb")
                pb_v = pb[:].rearrange("p (s mj r c) -> p s mj r c",
                                       s=8, mj=2, r=2, c=C)
                n_mms = len(_BANK_TILES[G])
                for j, (idx, t, l0, K, lastc) in enumerate(_BANK_TILES[G]):
                    si, o = _TILE_SLAB[idx]
                    st = get_slab(si)
                    if si + 1 < len(_SLAB_SIZES):
                        get_slab(si + 1)  # prefetch
                    s = t % 8
                    nc.tensor.matmul(
                        pb[:, s * 64:(s + 1) * 64],
                        st[0:K, o:o + 128],          # lhsT (K x [2m x 64k])
                        st[0:K, o + 128:o + 192],    # rhs (K x [2m x 2ri x 16c])
                        start=(j == 0), stop=(j == n_mms - 1))
                # extract diagonal (mi==mj) blocks -> snat (cast fp16),
                # split across DVE and ACT
                for mi in range(2):
                    for r in range(2):
                        eng = (nc.vector.tensor_copy if (mi + r) % 2 == 0
                               else nc.scalar.copy)
                        eng(snat_g[r * 64:(r + 1) * 64, g, :, mi, :],
                            pb_v[mi * 64:(mi + 1) * 64, :, mi, r, :])
                # interleave ~3 deferred units from the previous block
                for _ in range(3):
                    if deferred:
                        deferred.pop(0)()

            while deferred:
                deferred.pop(0)()
            deferred = make_deferred(b, snat_m, fsb_v)

        # last block's work has no next block to hide in
        while deferred:
            deferred.pop(0)()

    nc.compile()
    return nc


def _build_fmat():
    m = np.arange(M_E)
    n = np.arange(NLON)
    w = np.where(m == 0, 1.0, 2.0)
    ang = 2.0 * np.pi * np.outer(m, n) / NLON
    Fc = (w[:, None] * np.cos(ang)).astype(np.float16)
    Fs = (-w[:, None] * np.sin(ang)).astype(np.float16)
    fmat = np.zeros((128, NBLK * 2 * NLON), np.float16)
    for b in range(NBLK):
        fmat[:, b * 2 * NLON:b * 2 * NLON + NLON] = Fc[b * 128:(b + 1) * 128]
        fmat[:, b * 2 * NLON + NLON:(b + 1) * 2 * NLON] = Fs[b * 128:(b + 1) * 128]
    return fmat


def _pack_streams(x_re, x_im, pct):
    """Per-core packed fp16 stream: sequence of (128 x 192) tiles."""
    x_re = np.asarray(x_re, np.float32)
    x_im = np.asarray(x_im, np.float32)
    pct = np.asarray(pct, np.float32)

    # x part is core-independent: build once
    xpart = np.zeros((128, NTILES * 64), np.float16)
    for idx, (t, l0, K, _) in enumerate(TILES):
        xr = x_re[0, :, l0:l0 + K, 2 * t:2 * t + 2]   # (c, K, 2m)
        xi = x_im[0, :, l0:l0 + K, 2 * t:2 * t + 2]
        xx = np.stack([xr, xi], axis=0)                # (r, c, K, m)
        xpart[0:K, idx * 64:idx * 64 + 64] = \
            xx.transpose(2, 3, 0, 1).reshape(K, 64)

    streams = []
    for core in range(NCORES):
        k0 = core * KC
        sbuf = np.zeros((128, F_TOT), np.float16)
        sv = sbuf.reshape(128, NTILES, TILE_W)
        sv[:, :, 128:] = xpart.reshape(128, NTILES, 64)
        for idx, (t, l0, K, _) in enumerate(TILES):
            blk = pct[2 * t:2 * t + 2, k0:k0 + KC, l0:l0 + K]  # (2m, 64k, K)
            sv[0:K, idx, 0:128] = blk.transpose(2, 0, 1).reshape(K, 128)
        streams.append(sbuf)
    return streams


_NC_CACHE = [None]


def _get_program():
    if _NC_CACHE[0] is None:
        _NC_CACHE[0] = build_program()
    return _NC_CACHE[0]


def run(x_re, x_im, pct, nlon=NLON, trace=False, trace_kwargs=None):
    from concourse.bass_utils import run_bass_kernel_spmd

    assert int(nlon) == NLON
    nc = _get_program()
    fmat = _build_fmat()
    streams = _pack_streams(x_re, x_im, pct)
    in_maps = [{"stream": streams[i], "fmat": fmat} for i in range(NCORES)]
    res = run_bass_kernel_spmd(nc, in_maps, list(range(NCORES)),
                               trace=trace, **(trace_kwargs or {}))
    out = np.empty((B, C, NLAT, NLON), np.float32)
    for core in range(NCORES):
        yc = res.results[core]["y"].reshape(C, KC, NLON)
        out[0, :, core * KC:(core + 1) * KC, :] = yc
    return out, res


def kernel(x_re, x_im, pct, nlon=NLON, **_unused):
    out, _ = run(x_re, x_im, pct, nlon)
    return out
